# revision 6
# baseline (speedup 1.0000x reference)
"""AutoSparse forward kernel for Trainium2 (8 NeuronCores, SPMD).

Computes out = sign(W) * relu(|W| - sigmoid(threshold)) for
W: [4096, 8192] f32, threshold: [4096, 1] f32 (row-broadcast), via
out = w - clamp(w, -s, s).

Design (fast path, constant threshold; ~46.3-48.7us vs 49.8-51.4us for
the all-fp16 v1 baseline):

- fp16 input (host cast); ~82% of the OUTPUT stored as fp8 E3M4
  (4 mantissa bits). The ACT-engine fp16->e3m4 cast is bit-exact with
  ml_dtypes.float8_e3m4 (verified on HW); end-to-end rel err 1.233e-2
  vs the 2e-2 gate. Bytes drop 16.78 -> ~13.1 MB/core against a
  ~424 GB/s/core SBUF-AXI-port ceiling (16 SDMA engines x ~26.5 GB/s).
- Engine split: DVE cannot write fp8 (1-byte dtypes drop every DVE op
  to 1x mode), so DVE does clamp (tensor_scalar, 4x) + sub
  (tensor_tensor, 2x) in fp16 (28.7us busy, THE critical chain), and
  ACT casts to fp8 in parallel (ACTIVATE Copy, 1x @1.2GHz,
  (N+352)/1.2 ns). K16-col fp16 blocks per half + an all-fp16 tail
  half keep ACT's total below DVE's so casts never gate the tail.
- ALL loads dispatch first on the single Sync HWDGE ring: rings drain
  strictly FIFO per engine, so store descriptors can never interleave
  with (and stretch) the load phase. Stores on other rings DO steal
  load bandwidth (engines round-robin rings at packet granularity) --
  only the small K16 block stores ride the GpSimd ring. Note the tile
  framework SCHEDULES instructions; program order is only a hint for
  same-engine streams.
- The first three halves load and compute as 2048-col pieces so DVE's
  early lead builds without DMA-completion-receipt stalls (~1.5-2us
  per edge); later halves are whole 1MB loads.
- Tail: the last half is all-fp16 in two 2048-col piece-stores; the
  first piece dispatches as soon as its TT lands, so ~0.5MB of tail
  wire time overlaps DVE's final op, and both dispatch BEFORE the
  cast-gated last fp8 store (SDMA engine 15 intermittently runs
  ~16 GB/s on small-line stores and straggles the tail).
- Bass post-passes: split multi-wait instructions (walrus accepts one
  sync wait/op), strip the entry barrier, hoist wait-free load
  dispatches into the entry block, strip the exit epilogue's second
  drain round. Fixed costs: ~6.5us NEFF prologue, ~2us exit.

Sharding: rows split evenly across 8 cores (512 rows each); purely
elementwise per-row, so no collectives are needed. The general
per-row-threshold path (arbitrary threshold vectors) is the v1 fp16
kernel, unchanged.
"""

import numpy as np

import concourse.bass as bass
import concourse.tile as tile
from concourse import mybir
from concourse.bass_utils import run_bass_kernel_spmd

O, F = 4096, 8192
N_CORES = 8
ROWS = O // N_CORES          # 512 rows per core
P = 128                      # SBUF partitions
GROUPS = ROWS // P           # 4 row groups per core
HALF = F // 2                # 4096 cols per half
K16 = 256                    # fp16-stored cols at the head of each half
K8 = HALF - K16              # 3584 fp8-stored cols per half

_FP32 = mybir.dt.float32
_FP16 = mybir.dt.float16
_FP8 = mybir.dt.float8e3

_COPY = mybir.ActivationFunctionType.Copy


def _split_multi_waits(nc):
    """The walrus codegen in this container accepts at most ONE sync wait
    per instruction ("Too many sync wait commands"). Hoist all but the last
    wait of any multi-wait instruction into standalone same-engine
    InstEventSemaphore ops (the exact encoding raw-bass wait_ge uses)."""
    cnt = 0
    for fn in nc.m.functions:
        for b in fn.blocks:
            new = []
            for ins in b.instructions:
                si = ins.sync_info
                if si is not None and len(si.on_wait) > 1:
                    waits = list(si.on_wait)
                    for w in waits[:-1]:
                        cnt += 1
                        new.append(
                            mybir.InstEventSemaphore(
                                name=f"WSPLIT-{cnt}",
                                engine=ins.engine,
                                sync_info=mybir.SyncInfo(
                                    on_wait=[w], on_update=[]
                                ),
                            )
                        )
                    ins.sync_info = mybir.SyncInfo(
                        on_wait=[waits[-1]], on_update=list(si.on_update)
                    )
                new.append(ins)
            try:
                b.instructions = new
            except Exception:
                b.instructions[:] = new
    return nc


def _strip_entry_barrier(nc):
    """Drop the bass-emitted entry-block drains + barrier butterfly. The
    barrier's only purpose here is to order Pool const memsets against
    cross-engine readers; the kernel avoids framework const APs, so every
    remaining cross-engine dependency is already sem-carried."""
    b0 = nc.m.functions[0].blocks[0]
    keep = [
        ins
        for ins in b0.instructions
        if not (
            isinstance(ins, mybir.InstDrain)
            or (
                isinstance(ins, mybir.InstEventSemaphore)
                and ins.name.startswith("barrier_")
            )
        )
    ]
    try:
        b0.instructions = keep
    except Exception:
        b0.instructions[:] = keep
    return nc


def _strip_exit_round2(nc):
    """The bass epilogue runs TWO drain+barrier rounds; round 1 already
    orders everything. Drop everything after the InstISA marker."""
    bN = nc.m.functions[0].blocks[-1]
    ins_list = list(bN.instructions)
    isa_idx = next(
        (k for k, i in enumerate(ins_list) if isinstance(i, mybir.InstISA)),
        None,
    )
    if isa_idx is None:
        return nc
    tail = ins_list[isa_idx + 1 :]
    if not all(
        isinstance(i, (mybir.InstDrain, mybir.InstEventSemaphore)) for i in tail
    ):
        return nc
    keep = ins_list[: isa_idx + 1]
    try:
        bN.instructions = keep
    except Exception:
        bN.instructions[:] = keep
    return nc


def _early_first_loads(nc, limit=6):
    """Move the wait-free prefix of SP's body stream (the first weight
    loads) to the very top of SP's entry-block stream, ahead of the
    register moves. DMA copies carry static APs, so this is safe, and the
    BW-bound stream starts ~1.3us earlier."""
    fn = nc.m.functions[0]
    b0, b1 = fn.blocks[0], fn.blocks[1]
    for eng, lim in ((mybir.EngineType.SP, limit),):
        pre = []
        for ins in b1.instructions:
            if ins.engine != eng:
                continue
            si = ins.sync_info
            if (
                isinstance(ins, mybir.InstDMACopy)
                and (si is None or not si.on_wait)
                and len(pre) < lim
            ):
                pre.append(ins)
            else:
                break
        if not pre:
            continue
        body = [i for i in b1.instructions if i not in pre]
        entry = list(b0.instructions)
        idx = next(k for k, i in enumerate(entry) if i.engine == eng)
        entry[idx:idx] = pre
        try:
            b0.instructions = entry
            b1.instructions = body
        except Exception:
            b0.instructions[:] = entry
            b1.instructions[:] = body
    return nc


def _build_fast(s_const):
    """Constant-threshold kernel: clamp bounds are instruction immediates.

    Per-core layout (512 rows = 4 groups x 128 partitions, each group in
    two 4096-col halves, computed in load order):
      g0h0      two 2048-col chunks (early DVE start)
      g0h1..g3h0  fp8 halves: cols [0:K16) stored fp16 via the GpSimd
                ring, cols [K16:4096) ACT-cast to e3m4, stored on the
                Sync ring
      g3h1      all-fp16 tail: two TT pieces into ONE wide tile, ONE
                8KB-line 1MB store (narrow tail stores crawl at 2-4KB
                line rates, and an ACT cast here would trail DVE's end)

    ALL loads are dispatched first on the Sync ring: HWDGE rings drain
    strictly FIFO per engine, so every load descriptor drains before any
    Sync-ring store descriptor -- loads are never stretched by store
    interleaving (the engines round-robin BETWEEN rings at packet
    granularity, which is why the fp16 block stores ride the idle GpSimd
    ring only in small volume, and nothing rides the ACT ring at all).
    """
    nc = bass.Bass()
    w = nc.declare_dram_parameter("weight", [ROWS, F], _FP16, isOutput=False)
    o8 = nc.declare_dram_parameter("o8", [ROWS, F], _FP8, isOutput=True)
    o16 = nc.declare_dram_parameter(
        "o16", [ROWS, 2 * K16], _FP16, isOutput=True
    )
    o16b = nc.declare_dram_parameter("o16b", [P, HALF], _FP16, isOutput=True)

    s1, s2 = -s_const, s_const

    # the first three halves load (and compute) as 2048-col pieces: their
    # completion semaphores fire sooner, so DVE's early lead builds
    # without receipt-latency stalls
    loads = []
    for g, h in [(0, 0), (0, 1), (1, 0)]:
        loads += [(g, h, 0, 2048), (g, h, 2048, HALF)]
    loads += [
        (g, h, 0, HALF)
        for g, h in [(1, 1), (2, 0), (2, 1), (3, 0), (3, 1)]
    ]

    with tile.TileContext(nc) as tc:
        with (
            tc.tile_pool(name="w", bufs=len(loads)) as wp,
            tc.tile_pool(name="c", bufs=2) as cp,
            tc.tile_pool(name="t", bufs=6) as tp,
            tc.tile_pool(name="tt", bufs=2) as ttp,
            tc.tile_pool(name="e", bufs=9) as ep,
        ):
            wts = []
            for g, h, c0, c1 in loads:
                t_ = wp.tile([P, c1 - c0], _FP16)
                nc.sync.dma_start(
                    out=t_,
                    in_=w[g * P : (g + 1) * P, h * HALF + c0 : h * HALF + c1],
                )
                wts.append(t_)

            def clamp_sub(wt, base, c0, c1, out=None):
                n = c1 - c0
                ct = cp.tile([P, n], _FP16)
                nc.vector.tensor_scalar(
                    out=ct,
                    in0=wt[:, c0 - base : c1 - base],
                    scalar1=s1,
                    scalar2=s2,
                    op0=mybir.AluOpType.max,
                    op1=mybir.AluOpType.min,
                )
                if out is None:
                    out = tp.tile([P, n], _FP16)
                nc.vector.tensor_sub(out, wt[:, c0 - base : c1 - base], ct)
                return out

            g3h0 = None
            for wt, (g, h, c0, c1) in zip(wts, loads):
                rows = slice(g * P, (g + 1) * P)
                if g == GROUPS - 1 and h == 1:
                    # two piece-stores instead of one wide store: the
                    # first 0.5MB drains DURING the final TT (its store
                    # dispatches as soon as TT1 lands), so only ~0.5MB of
                    # tail fp16 remains after DVE's last op
                    otB1 = ttp.tile([P, 2048], _FP16)
                    clamp_sub(wt, 0, 0, 2048, out=otB1)
                    nc.sync.dma_start(out=o16b[:, 0:2048], in_=otB1)
                    otB2 = ttp.tile([P, 2048], _FP16)
                    clamp_sub(wt, 0, 2048, HALF, out=otB2)
                    nc.sync.dma_start(out=o16b[:, 2048:HALF], in_=otB2)
                    # g3h0's cast-gated fp8 store dispatches AFTER the wide
                    # tail store in Sync's program, so the 8KB-line 1MB
                    # store isn't FIFO-blocked behind the cast gate
                    ot30, e30 = g3h0
                    nc.sync.dma_start(
                        out=o8[rows, h0r * HALF + K16 : (h0r + 1) * HALF],
                        in_=e30,
                    )
                    continue
                ot = clamp_sub(wt, c0, c0, c1)
                k16 = K16 if c0 < K16 else 0
                if k16:
                    nc.gpsimd.dma_start(
                        out=o16[rows, h * K16 : (h + 1) * K16],
                        in_=ot[:, 0 : K16 - c0],
                    )
                f0 = max(c0, k16)
                e8 = ep.tile([P, c1 - f0], _FP8)
                nc.scalar.activation(
                    out=e8, in_=ot[:, f0 - c0 : c1 - c0], func=_COPY
                )
                if g == GROUPS - 1 and h == 0:
                    g3h0, h0r = (ot, e8), h
                    continue
                nc.sync.dma_start(
                    out=o8[rows, h * HALF + f0 : h * HALF + c1], in_=e8
                )

    return _strip_exit_round2(
        _early_first_loads(
            _strip_entry_barrier(_split_multi_waits(nc)), limit=12
        )
    )


# ---------------------------------------------------------------------------
# General per-row-threshold path (from the v1 baseline, fp16 in/out).
# ---------------------------------------------------------------------------
COL_TILE = 8192


def _build_general():
    nc = bass.Bass()
    w = nc.declare_dram_parameter("weight", [ROWS, F], _FP16, isOutput=False)
    sc = nc.declare_dram_parameter(
        "scales", [P, 2 * GROUPS], _FP32, isOutput=False
    )
    out = nc.declare_dram_parameter("out", [ROWS, F], _FP16, isOutput=True)

    with tile.TileContext(nc) as tc:
        with (
            tc.tile_pool(name="const", bufs=1) as constp,
            tc.tile_pool(name="w", bufs=3) as wp,
            tc.tile_pool(name="c", bufs=3) as cp,
            tc.tile_pool(name="o", bufs=3) as op,
        ):
            sct = constp.tile([P, 2 * GROUPS], _FP32)
            nc.scalar.dma_start(out=sct, in_=sc[:, :])
            s = sct[:, 0:GROUPS]
            ns = sct[:, GROUPS : 2 * GROUPS]
            warm = constp.tile([P, 1], _FP32)
            nc.vector.tensor_scalar(
                out=warm,
                in0=s[:, 0:1],
                scalar1=ns[:, 0:1],
                scalar2=None,
                op0=mybir.AluOpType.add,
            )
            for g in range(GROUPS):
                rows = slice(g * P, (g + 1) * P)
                s1, s2 = ns[:, g : g + 1], s[:, g : g + 1]
                wt = wp.tile([P, COL_TILE], _FP16)
                nc.sync.dma_start(out=wt, in_=w[rows, :])
                ct = cp.tile([P, COL_TILE], _FP16)
                nc.vector.tensor_scalar(
                    out=ct,
                    in0=wt,
                    scalar1=s1,
                    scalar2=s2,
                    op0=mybir.AluOpType.max,
                    op1=mybir.AluOpType.min,
                )
                ot = op.tile([P, COL_TILE], _FP16)
                nc.vector.tensor_sub(ot, wt, ct)
                nc.scalar.dma_start(out=out[rows, :], in_=ot)
    return _strip_exit_round2(
        _early_first_loads(_strip_entry_barrier(_split_multi_waits(nc)))
    )


_nc_cache = {}


def _get_nc(key, builder):
    if key not in _nc_cache:
        _nc_cache[key] = builder()
    return _nc_cache[key]


def kernel(weight, threshold, trace=False):
    weight = np.asarray(weight)
    threshold = np.ascontiguousarray(np.asarray(threshold, dtype=np.float32))
    assert weight.shape == (O, F) and threshold.shape == (O, 1)
    w16 = np.ascontiguousarray(weight.astype(np.float16))
    s_all = (1.0 / (1.0 + np.exp(-threshold.astype(np.float64)))).astype(
        np.float32
    )

    fast = bool(np.all(threshold == threshold.flat[0]))
    if fast:
        s_const = float(s_all.flat[0])
        nc = _get_nc(("fast", s_const), lambda: _build_fast(s_const))
        in_maps = [
            {"weight": w16[i * ROWS : (i + 1) * ROWS]} for i in range(N_CORES)
        ]
    else:
        nc = _get_nc(("gen",), _build_general)
        in_maps = []
        for i in range(N_CORES):
            s_shard = s_all[i * ROWS : (i + 1) * ROWS].reshape(GROUPS, P).T
            in_maps.append(
                {
                    "weight": w16[i * ROWS : (i + 1) * ROWS],
                    "scales": np.ascontiguousarray(
                        np.concatenate([s_shard, -s_shard], axis=1)
                    ),
                }
            )
    kwargs = {}
    if trace:
        import os

        tdir = os.path.abspath("trace_out")
        os.makedirs(tdir, exist_ok=True)
        for f in os.listdir(tdir):
            os.remove(os.path.join(tdir, f))
        os.environ["KEEP_NEFF_DIR"] = tdir
        kwargs["tmpdir"] = tdir
    res = run_bass_kernel_spmd(
        nc, in_maps, list(range(N_CORES)), trace=trace, **kwargs
    )
    full = np.empty((O, F), dtype=np.float32)
    for i in range(N_CORES):
        rows = slice(i * ROWS, (i + 1) * ROWS)
        if fast:
            r8 = res.results[i]["o8"].astype(np.float32)      # [ROWS, F]
            r16 = res.results[i]["o16"].astype(np.float32)    # [ROWS, 2*K16]
            rb = res.results[i]["o16b"].astype(np.float32)    # [P, HALF]
            sh = full[rows]
            sh[:, :] = r8
            for g, h in [(0, 0), (0, 1), (1, 0), (1, 1), (2, 0), (2, 1),
                         (3, 0)]:
                rr = slice(g * P, (g + 1) * P)
                sh[rr, h * HALF : h * HALF + K16] = r16[
                    rr, h * K16 : h * K16 + K16
                ]
            sh[3 * P :, HALF:] = rb                           # g3 h1 (fp16)
            full[rows] = sh
        else:
            full[rows] = res.results[i]["out"].astype(np.float32)
    if trace:
        return full, res
    return full
